# revision 3
# baseline (speedup 1.0000x reference)
"""Trainium2 Bass kernel for nn_Attention_2087354105914 (sparse_attention).

Strategy: pure data-parallel over batch (32 images / 8 cores = 4 per core, no
collectives). Per core:
  - q = 1x1 conv (PE matmul, scale 1/8 folded into weights)
  - spatial-reduction path: depthwise 4x4/s4 conv + BN+ReLU + per-channel
    affine + depthwise 3x3 conv + residual, convs done as diagonal-matmul
    PSUM accumulation on PE
  - kv projection (PE matmul), v transposed per head via PE transpose
  - attention in S^T layout [Nk, N]: exp on ScalarE straight out of PSUM,
    AV matmul with stationary [v.T | ones] so PSUM rows 0:64 hold Y and rows
    64:128 hold the softmax denominator; normalize = DVE reciprocal + mult.
"""
import sys
import numpy as np

sys.path.insert(0, "/opt/trn_rl_repo")

B, C, H, W = 32, 128, 56, 56
N = H * W            # 3136
NB = 4               # batches per core
NCORES = 8
NK = 225             # 15*15 reduced spatial
BN_EPS = 1e-5

NTILE = 512
SPAN = 1024
# n tiles: 6x512 + 64
NT = [(i * NTILE, min((i + 1) * NTILE, N)) for i in range((N + NTILE - 1) // NTILE)]
SPANS = [(i * SPAN, min((i + 1) * SPAN, N)) for i in range((N + SPAN - 1) // SPAN)]

_NC = None


def _build(nb=NB, normalize="crossbase"):
    import concourse.bacc as bacc
    import concourse.tile as tile
    from concourse import mybir

    f32 = mybir.dt.float32
    bf16 = mybir.dt.bfloat16
    AF = mybir.ActivationFunctionType
    OP = mybir.AluOpType

    nc = bacc.Bacc("TRN2", target_bir_lowering=False, debug=False)

    x_d = nc.dram_tensor("x", [nb, C, N], f32, kind="ExternalInput")
    qwT_d = nc.dram_tensor("q_wT", [C, C], f32, kind="ExternalInput")
    qb_d = nc.dram_tensor("q_b2", [C, 1], f32, kind="ExternalInput")
    kvwT_d = nc.dram_tensor("kv_wT", [C, 2 * C], f32, kind="ExternalInput")
    kb_d = nc.dram_tensor("k_b", [C, 1], f32, kind="ExternalInput")
    vb_d = nc.dram_tensor("v_b", [C, 1], f32, kind="ExternalInput")
    b1_d = nc.dram_tensor("b1", [C, 1], f32, kind="ExternalInput")
    a2_d = nc.dram_tensor("a2", [C, 1], f32, kind="ExternalInput")
    b2_d = nc.dram_tensor("b2", [C, 1], f32, kind="ExternalInput")
    w1_d = nc.dram_tensor("w1", [C, 16], f32, kind="ExternalInput")
    wl_d = nc.dram_tensor("wl", [C, 9], f32, kind="ExternalInput")
    lb_d = nc.dram_tensor("lb", [C, 1], f32, kind="ExternalInput")
    eye_d = nc.dram_tensor("eye", [C, C], f32, kind="ExternalInput")
    out_d = nc.dram_tensor("out", [nb, C, N], f32, kind="ExternalOutput")

    with tile.TileContext(nc) as tc:
        import contextlib

        with contextlib.ExitStack() as ctx:
            consts = ctx.enter_context(tc.tile_pool(name="consts", bufs=1))
            xqp = ctx.enter_context(tc.tile_pool(name="xq", bufs=2))
            xpp = ctx.enter_context(tc.tile_pool(name="xpad", bufs=2))
            qp = ctx.enter_context(tc.tile_pool(name="q", bufs=2))
            kvp = ctx.enter_context(tc.tile_pool(name="kv", bufs=2))
            vtp = ctx.enter_context(tc.tile_pool(name="vt", bufs=2))
            ep = ctx.enter_context(tc.tile_pool(name="e", bufs=2))
            ohp = ctx.enter_context(tc.tile_pool(name="oh", bufs=2))
            rp = ctx.enter_context(tc.tile_pool(name="r", bufs=2))
            ps = ctx.enter_context(tc.tile_pool(name="ps", bufs=2, space="PSUM"))

            def cload(dram, shape, dtype=f32):
                t = consts.tile(shape, dtype, tag=f"c_{dram.name}")
                nc.sync.dma_start(t[:], dram[:])
                return t

            qwT = cload(qwT_d, [C, C])
            qb = cload(qb_d, [C, 1])
            kvwT = cload(kvwT_d, [C, 2 * C])
            kb = cload(kb_d, [C, 1])
            vb = cload(vb_d, [C, 1])
            b1 = cload(b1_d, [C, 1])
            a2 = cload(a2_d, [C, 1])
            b2 = cload(b2_d, [C, 1])
            w1 = cload(w1_d, [C, 16])
            wl = cload(wl_d, [C, 9])
            lb = cload(lb_d, [C, 1])
            eye = cload(eye_d, [C, C])

            # diagonal weight matrices for depthwise convs
            diag1 = consts.tile([C, 16, C], f32)
            for t in range(16):
                nc.vector.tensor_scalar_mul(diag1[:, t, :], eye[:], w1[:, t : t + 1])
            diagl = consts.tile([C, 9, C], f32)
            for t in range(9):
                nc.vector.tensor_scalar_mul(diagl[:, t, :], eye[:], wl[:, t : t + 1])

            for b in range(nb):
                # ---- load x ----
                xq = xqp.tile([C, N], f32, tag="xq")
                nc.sync.dma_start(xq[:], x_d[b])
                xp = xpp.tile([C, 60, 60], f32, tag="xp")
                nc.gpsimd.memset(xp[:, 0:2, :], 0.0)
                nc.gpsimd.memset(xp[:, 58:60, :], 0.0)
                nc.gpsimd.memset(xp[:, 2:58, 0:2], 0.0)
                nc.gpsimd.memset(xp[:, 2:58, 58:60], 0.0)
                nc.gpsimd.tensor_copy(
                    xp[:, 2:58, 2:58], xq[:].rearrange("p (h w) -> p h w", h=H)
                )

                # ---- q projection ----
                q_sb = qp.tile([C, N], bf16, tag="q")
                for (n0, n1) in NT:
                    qps = ps.tile([C, SPAN], f32, tag="s")
                    nc.tensor.matmul(
                        qps[:, : n1 - n0], qwT[:], xq[:, n0:n1], start=True, stop=True
                    )
                    nc.vector.tensor_scalar_add(
                        q_sb[:, n0:n1], qps[:, : n1 - n0], qb[:]
                    )

                # ---- spatial reduction conv (4x4 stride 4, BN+ReLU folded) ----
                kvps = ps.tile([C, 15, 15], f32, tag="s")
                for t in range(16):
                    kh, kw = divmod(t, 4)
                    nc.tensor.matmul(
                        kvps[:],
                        diag1[:, t, :],
                        xp[:, kh : kh + 57 : 4, kw : kw + 57 : 4],
                        start=(t == 0),
                        stop=(t == 15),
                    )
                kv1r = kvp.tile([C, 15, 15], f32, tag="kv1r")
                nc.scalar.activation(kv1r[:], kvps[:], AF.Relu, bias=b1[:], scale=1.0)

                # sr2 affine into padded tile for local conv
                kv2p = kvp.tile([C, 17, 17], f32, tag="kv2p")
                nc.gpsimd.memset(kv2p[:], 0.0)
                nc.vector.tensor_scalar(
                    kv2p[:, 1:16, 1:16],
                    kv1r[:],
                    a2[:],
                    b2[:],
                    op0=OP.mult,
                    op1=OP.add,
                )

                # ---- local 3x3 depthwise conv + bias + residual ----
                kv3ps = ps.tile([C, 15, 15], f32, tag="s")
                for t in range(9):
                    kh, kw = divmod(t, 3)
                    nc.tensor.matmul(
                        kv3ps[:],
                        diagl[:, t, :],
                        kv2p[:, kh : kh + 15, kw : kw + 15],
                        start=(t == 0),
                        stop=(t == 8),
                    )
                kv3 = kvp.tile([C, 15, 15], f32, tag="kv3")
                nc.vector.scalar_tensor_tensor(
                    kv3[:], kv3ps[:], lb[:], kv2p[:, 1:16, 1:16], op0=OP.add, op1=OP.add
                )

                # ---- kv projection ----
                k_sb = kvp.tile([C, NK], bf16, tag="k")
                v_sb = kvp.tile([C, NK], f32, tag="v")
                kps = ps.tile([C, NK], f32, tag="s")
                nc.tensor.matmul(kps[:], kvwT[:, 0:C], kv3[:], start=True, stop=True)
                nc.vector.tensor_scalar_add(k_sb[:], kps[:], kb[:])
                vps = ps.tile([C, NK], f32, tag="s")
                nc.tensor.matmul(vps[:], kvwT[:, C : 2 * C], kv3[:], start=True, stop=True)
                nc.vector.tensor_scalar_add(v_sb[:], vps[:], vb[:])

                for h in range(2):
                    hs = 64 * h
                    # ---- build [v.T | ones] stationary tiles per m-chunk ----
                    vt0 = vtp.tile([C, C], bf16, tag="vt0")
                    vt1 = vtp.tile([C, C], bf16, tag="vt1")
                    tp0 = ps.tile([C, 64], f32, tag="s")
                    nc.tensor.transpose(
                        tp0[:], v_sb[hs : hs + 64, 0:128], eye[hs : hs + 64, hs : hs + 64]
                    )
                    nc.vector.tensor_copy(vt0[:, 0:64], tp0[:])
                    nc.gpsimd.memset(vt0[:, 64:128], 1.0)
                    tp1 = ps.tile([C, 64], f32, tag="s")
                    nc.tensor.transpose(
                        tp1[0:97, :], v_sb[hs : hs + 64, 128:225], eye[hs : hs + 64, hs : hs + 64]
                    )
                    nc.vector.tensor_copy(vt1[0:97, 0:64], tp1[0:97, :])
                    nc.gpsimd.memset(vt1[0:97, 64:128], 1.0)

                    # ---- S^T = k.T q per m-chunk, exp on ScalarE ----
                    e0 = ep.tile([C, N], bf16, tag="e0")
                    e1 = ep.tile([C, N], bf16, tag="e1")
                    for ci, (m0, m1) in enumerate(((0, 128), (128, 225))):
                        csz = m1 - m0
                        etile = e0 if ci == 0 else e1
                        for (s0, s1) in SPANS:
                            sps = ps.tile([C, SPAN], f32, tag="s")
                            for (n0, n1) in NT:
                                if n0 < s0 or n0 >= s1:
                                    continue
                                nc.tensor.matmul(
                                    sps[0:csz, n0 - s0 : n1 - s0],
                                    k_sb[hs : hs + 64, m0:m1],
                                    q_sb[hs : hs + 64, n0:n1],
                                    start=True,
                                    stop=True,
                                )
                            nc.scalar.activation(
                                etile[0:csz, s0:s1], sps[0:csz, 0 : s1 - s0], AF.Exp
                            )

                    # ---- AV + denominator + normalize ----
                    out_h = ohp.tile([64, N], f32, tag="oh")
                    for (s0, s1) in SPANS:
                        w_ = s1 - s0
                        yps = ps.tile([C, SPAN], f32, tag="y")
                        for (n0, n1) in NT:
                            if n0 < s0 or n0 >= s1:
                                continue
                            nc.tensor.matmul(
                                yps[:, n0 - s0 : n1 - s0],
                                vt0[:],
                                e0[:, n0:n1],
                                start=True,
                                stop=False,
                            )
                            nc.tensor.matmul(
                                yps[:, n0 - s0 : n1 - s0],
                                vt1[0:97, :],
                                e1[0:97, n0:n1],
                                start=False,
                                stop=True,
                            )
                        r = rp.tile([C, SPAN], f32, tag="r")
                        nc.vector.reciprocal(r[64:128, 0:w_], yps[64:128, 0:w_])
                        if normalize == "crossbase":
                            nc.vector.tensor_mul(
                                out_h[:, s0:s1], yps[0:64, 0:w_], r[64:128, 0:w_]
                            )
                        else:
                            rb = rp.tile([64, SPAN], f32, tag="rb")
                            nc.gpsimd.partition_broadcast(
                                rb[:, 0:w_], r[64:65, 0:w_], channels=64
                            )
                            nc.vector.tensor_mul(
                                out_h[:, s0:s1], yps[0:64, 0:w_], rb[:, 0:w_]
                            )
                    nc.sync.dma_start(out_d[b, hs : hs + 64, :], out_h[:])

    nc.compile()
    return nc


def _get_nc():
    global _NC
    if _NC is None:
        _NC = _build()
    return _NC


def _host_consts(inputs):
    f = lambda k: np.asarray(inputs[k], np.float32)
    q_w, q_b = f("q_w"), f("q_b")
    kv_w, kv_b = f("kv_w"), f("kv_b")
    sr1_w, sr1_g, sr1_b, sr1_m, sr1_v = (
        f("sr1_w"), f("sr1_g"), f("sr1_b"), f("sr1_m"), f("sr1_v"))
    sr2_w, sr2_g, sr2_b, sr2_m, sr2_v = (
        f("sr2_w"), f("sr2_g"), f("sr2_b"), f("sr2_m"), f("sr2_v"))
    local_w, local_b = f("local_w"), f("local_b")

    scale = (C // 2) ** -0.5  # d = 64
    a1 = sr1_g / np.sqrt(sr1_v + BN_EPS)
    b1 = sr1_b - sr1_m * a1
    a2r = sr2_g / np.sqrt(sr2_v + BN_EPS)
    a2 = a2r * sr2_w.reshape(C)
    b2 = sr2_b - sr2_m * a2r

    consts = {
        "q_wT": np.ascontiguousarray(q_w.T) * scale,
        "q_b2": (q_b * scale).reshape(C, 1),
        "kv_wT": np.ascontiguousarray(kv_w.T),
        "k_b": kv_b[0:C].reshape(C, 1),
        "v_b": kv_b[C : 2 * C].reshape(C, 1),
        "b1": b1.reshape(C, 1),
        "a2": a2.reshape(C, 1),
        "b2": b2.reshape(C, 1),
        "w1": np.ascontiguousarray(sr1_w.reshape(C, 16) * a1[:, None]),
        "wl": np.ascontiguousarray(local_w.reshape(C, 9)),
        "lb": local_b.reshape(C, 1),
        "eye": np.eye(C, dtype=np.float32),
    }
    return {k: np.ascontiguousarray(v, dtype=np.float32) for k, v in consts.items()}


def kernel(**inputs):
    from concourse.bass_utils import run_bass_kernel_spmd

    x = np.ascontiguousarray(np.asarray(inputs["x"], np.float32)).reshape(B, C, N)
    consts = _host_consts(inputs)
    nc = _get_nc()
    in_maps = []
    for c in range(NCORES):
        m = {"x": np.ascontiguousarray(x[c * NB : (c + 1) * NB])}
        m.update(consts)
        in_maps.append(m)
    res = run_bass_kernel_spmd(nc, in_maps, core_ids=list(range(NCORES)))
    out = np.concatenate([res.results[i]["out"] for i in range(NCORES)], axis=0)
    return out.reshape(B, C, H, W)
